# revision 1
# baseline (speedup 1.0000x reference)
"""MVS plane-sweep cost-volume kernel for Trainium2 (Bass/Tile), 8 NeuronCores.

Strategy (v2, SWDGE dma_gather):
  - 8 (batch, view) pairs -> 8 cores (data-parallel over the view loop).
  - Host computes the warp fields exactly as the reference does and builds,
    per (b,v): a corner-packed padded image Z4 in DRAM where row q holds
    [fea(:,q), fea(:,q+1), fea(:,q+W), fea(:,q+W+1)] (128 f32 = 512 B), a
    single int16 gather row-index per (depth, pixel), and 4 folded corner
    weights (bilinear * valid * view_weight / C).
  - Device (per core, per depth-plane, per 2048-pixel chunk): one SWDGE
    dma_gather pulls 2048 x 512B corner-packs from HBM into SBUF in
    pixel-partitioned layout [128 pix, 16, 4*32]; DVE multiplies by the
    (resident, pre-transposed+replicated) ref features, tensor_reduce(X)
    sums over C, multiplies by corner weights, tensor_reduce(X) sums the
    4 corners -> [128 pix, 16] plane-chunk output.  No TensorE needed.
  - Host: un-permute, sum partials over the 4 views, divide by weight sum.

Self-contained: shapes hardcoded for the nn_DI_MVS problem instance.
"""
import numpy as np

B, V, C, H, W = 2, 5, 32, 128, 160
D = 48
HW = H * W
NCORES = 8
CHUNK = 2048             # pixels per dma_gather call
NCHUNKS = HW // CHUNK    # 10
PAD = W + 1              # index shift so clamped corner bases stay >= 0
NZ = HW + W + 1          # padded Z4 row count
ELEM = 4 * C             # 128 values per gathered row
DATA_F16 = False         # gather/ref data dtype (f16 halves HBM traffic)

_PROGRAM_CACHE = {}


# ----------------------------------------------------------------- host math
def _fold(proj):
    out = proj[0].copy()
    out[:3, :4] = (proj[1][:3, :3] @ proj[0][:3, :4]).astype(np.float32)
    return out


def _host_fields(features, proj_matrices, depth_values, view_weights):
    """Per (b,v) core: corner-base gather index + 4 folded corner weights."""
    ys, xs = np.meshgrid(np.arange(H, dtype=np.float32),
                         np.arange(W, dtype=np.float32), indexing='ij')
    grid = np.stack([xs.ravel(), ys.ravel(), np.ones(HW, dtype=np.float32)], 0)

    cores = []
    for b in range(B):
        ref_p = _fold(proj_matrices[b, 0])
        ref_p_inv = np.linalg.inv(ref_p.astype(np.float64)).astype(np.float32)
        for v in range(1, V):
            proj = (_fold(proj_matrices[b, v]).astype(np.float64)
                    @ ref_p_inv.astype(np.float64)).astype(np.float32)
            rot, trans = proj[:3, :3], proj[:3, 3]
            rot_xyz = rot.astype(np.float32) @ grid
            dep = depth_values[b].astype(np.float32)
            pxyz = (rot_xyz[:, None, :] * dep[None, :, None]
                    + trans[:, None, None]).astype(np.float32)
            px = (pxyz[0] / pxyz[2]).astype(np.float32)
            py = (pxyz[1] / pxyz[2]).astype(np.float32)
            x0 = np.floor(px)
            y0 = np.floor(py)
            wx = px - x0
            wy = py - y0
            vw = view_weights[b, v - 1].reshape(HW)

            # corner-base row index into the padded Z4 image
            x0c = np.clip(x0, -1, W - 1)
            y0c = np.clip(y0, -1, H - 1)
            idx = (y0c * W + x0c + PAD).astype(np.int32)     # (D, HW) in [0, NZ)

            wt4 = np.empty((4, D, HW), dtype=np.float32)
            corners = [(x0, y0, (1 - wx) * (1 - wy)),
                       (x0 + 1, y0, wx * (1 - wy)),
                       (x0, y0 + 1, (1 - wx) * wy),
                       (x0 + 1, y0 + 1, wx * wy)]
            for k, (xi, yi, wk) in enumerate(corners):
                valid = ((xi >= 0) & (xi <= W - 1) & (yi >= 0) & (yi <= H - 1))
                # a clamped base shifts which Z4 slot holds the corner's value;
                # those corners always have weight 0, so slot mismatch is fine.
                wt4[k] = (wk * valid).astype(np.float32) * vw[None, :] / np.float32(C)
            cores.append((b, v, idx, wt4))
    return cores


def _build_z4(src):
    """src: (C, HW) f32 -> padded corner-packed image (NZ, 4C) f32."""
    q = np.arange(NZ, dtype=np.int64) - PAD
    z4 = np.empty((NZ, 4, C), dtype=np.float32)
    for s, off in enumerate((0, 1, W, W + 1)):
        qi = np.clip(q + off, 0, HW - 1)
        z4[:, s, :] = src[:, qi].T
    return z4.reshape(NZ, 4 * C)


def _pack_core_inputs(features, cores):
    in_maps = []
    for (b, v, idx, wt4) in cores:
        src = features[b, v].reshape(C, HW).astype(np.float32)
        ref = features[b, 0].reshape(C, HW).astype(np.float32)
        dt = np.float16 if DATA_F16 else np.float32
        z4 = _build_z4(src).astype(dt)            # (NZ, 128)
        # ref transposed + duplicated x4 corners: (HW, 128)
        reft4 = np.tile(ref.T, (1, 4)).astype(dt)

        # idx tensor [D, NCHUNKS, 128, CHUNK//16] int16: wrapped in 16
        # partitions (j%16, j//16) and replicated to all 8 cores.
        blk = idx.reshape(D, NCHUNKS, CHUNK // 16, 16).astype(np.int16)
        wrap = blk.transpose(0, 1, 3, 2)          # (D, NCHUNKS, 16, 128)
        idx_t = np.tile(wrap, (1, 1, 8, 1))       # (D, NCHUNKS, 128, 128)

        # wts tensor [D, NCHUNKS, 128, 16*4]: [pixel%128, (i, corner)]
        wt_t = (wt4.transpose(1, 2, 0)            # (D, HW, 4)
                .reshape(D, NCHUNKS, 16, 128, 4)  # (d, ch, i, p, k)
                .transpose(0, 1, 3, 2, 4)         # (d, ch, p, i, k)
                .reshape(D, NCHUNKS, 128, 64)
                .astype(np.float32))

        in_maps.append({
            "z4": z4,
            "reft4": reft4,
            "idx": np.ascontiguousarray(idx_t),
            "wts": np.ascontiguousarray(wt_t),
        })
    return in_maps


# ------------------------------------------------------------- bass program
def _build_program():
    import concourse.bacc as bacc
    import concourse.tile as tile
    import concourse.mybir as mybir

    nc = bacc.Bacc("TRN2", target_bir_lowering=False, debug=False,
                   num_devices=NCORES, num_swdge_queues=4)
    f32 = mybir.dt.float32
    f16 = mybir.dt.float16 if DATA_F16 else mybir.dt.float32
    i16 = mybir.dt.int16

    z4_d = nc.dram_tensor("z4", [NZ, ELEM], f16, kind="ExternalInput")
    reft4_d = nc.dram_tensor("reft4", [HW, ELEM], f16, kind="ExternalInput")
    idx_d = nc.dram_tensor("idx", [D, NCHUNKS, 128, CHUNK // 16], i16,
                           kind="ExternalInput")
    wts_d = nc.dram_tensor("wts", [D, NCHUNKS, 128, 64], f32,
                           kind="ExternalInput")
    out_d = nc.dram_tensor("out", [D, NCHUNKS, 128, 16], f32,
                           kind="ExternalOutput")

    with tile.TileContext(nc) as tc:
        with (
            tc.tile_pool(name="big", bufs=1) as big,
            tc.tile_pool(name="gat", bufs=6) as gat,
            tc.tile_pool(name="idxp", bufs=8) as idxp,
            tc.tile_pool(name="wtp", bufs=8) as wtp,
            tc.tile_pool(name="crp", bufs=6) as crp,
            tc.tile_pool(name="outp", bufs=6) as outp,
        ):
            # resident ref: [128, NCHUNKS*16*128] with dst[p, (ch*16+i)*128+c]
            # = reft4[ch*2048 + i*128 + p, c]
            refsb = big.tile([128, HW // 128 * ELEM], f16)
            ref_src = reft4_d.ap().rearrange("(blk p) e -> p blk e", p=128)
            nc.sync.dma_start(
                refsb[:].rearrange("p (blk e) -> p blk e", e=ELEM), ref_src
            )

            z4_ap = z4_d.ap()
            gq = 0

            for d in range(D):
                for ch in range(NCHUNKS):
                    idxt = idxp.tile([128, CHUNK // 16], i16)
                    nc.sync.dma_start(idxt[:], idx_d.ap()[d, ch])
                    wtt = wtp.tile([128, 64], f32)
                    nc.sync.dma_start(wtt[:], wts_d.ap()[d, ch])

                    g = gat.tile([128, (CHUNK // 128) * ELEM], f16)
                    # SWDGE descriptor ring holds 1024 descs; split the
                    # 2048-row gather into two 1024-row calls.
                    half = CHUNK // 2
                    for h in range(2):
                        nc.gpsimd.dma_gather(
                            g[:, h * (half // 128) * ELEM:
                              (h + 1) * (half // 128) * ELEM]
                            .rearrange("p (i e) -> p i e", e=ELEM),
                            z4_ap,
                            idxt[:, h * (half // 16):(h + 1) * (half // 16)],
                            num_idxs=half,
                            num_idxs_reg=half,
                            elem_size=ELEM,
                            queue_num=gq % 4,
                        )
                        gq += 1
                    nc.vector.tensor_mul(
                        g[:], g[:],
                        refsb[:, ch * (CHUNK // 128) * ELEM:
                              (ch + 1) * (CHUNK // 128) * ELEM],
                    )
                    cr = crp.tile([128, 64], f32)
                    nc.vector.tensor_reduce(
                        cr[:],
                        g[:].rearrange("p (s c) -> p s c", c=C),
                        axis=mybir.AxisListType.X,
                        op=mybir.AluOpType.add,
                    )
                    nc.vector.tensor_mul(cr[:], cr[:], wtt[:])
                    outt = outp.tile([128, 16], f32)
                    nc.vector.tensor_reduce(
                        outt[:],
                        cr[:].rearrange("p (i k) -> p i k", k=4),
                        axis=mybir.AxisListType.X,
                        op=mybir.AluOpType.add,
                    )
                    nc.sync.dma_start(out_d.ap()[d, ch], outt[:])

    nc.compile()
    return nc


def _get_program():
    if "nc" not in _PROGRAM_CACHE:
        _PROGRAM_CACHE["nc"] = _build_program()
    return _PROGRAM_CACHE["nc"]


# -------------------------------------------------------------------- runner
def _run(inputs, trace=False):
    from concourse.bass_utils import run_bass_kernel_spmd

    features = np.asarray(inputs["features"], dtype=np.float32)
    proj_matrices = np.asarray(inputs["proj_matrices"], dtype=np.float32)
    depth_values = np.asarray(inputs["depth_values"], dtype=np.float32)
    view_weights = np.asarray(inputs["view_weights"], dtype=np.float32)

    cores = _host_fields(features, proj_matrices, depth_values, view_weights)
    in_maps = _pack_core_inputs(features, cores)
    nc = _get_program()

    res = run_bass_kernel_spmd(nc, in_maps, core_ids=list(range(NCORES)),
                               trace=trace)
    # out [D, NCHUNKS, 128, 16] -> [D, HW] with pixel = ch*2048 + i*128 + p
    partials = [
        res.results[i]["out"].transpose(0, 1, 3, 2).reshape(D, HW)
        for i in range(NCORES)
    ]

    out = np.empty((B, 1, D, H, W), dtype=np.float32)
    for b in range(B):
        vol = np.zeros((D, HW), dtype=np.float32)
        wsum = np.full((HW,), 1e-5, dtype=np.float32)
        for v in range(1, V):
            vol = vol + partials[b * 4 + (v - 1)]
            wsum = wsum + view_weights[b, v - 1].reshape(HW)
        out[b, 0] = (vol / wsum[None, :]).reshape(D, H, W)
    return out, res


def kernel(**inputs) -> np.ndarray:
    out, _ = _run(inputs, trace=False)
    return out



# revision 8
# speedup vs baseline: 1.0268x; 1.0268x over previous
"""MVS plane-sweep cost-volume kernel for Trainium2 (Bass/Tile), 8 NeuronCores.

Strategy (v2, SWDGE dma_gather):
  - 8 (batch, view) pairs -> 8 cores (data-parallel over the view loop).
  - Host computes the warp fields exactly as the reference does and builds,
    per (b,v): a corner-packed padded image Z4 in DRAM where row q holds
    [fea(:,q), fea(:,q+1), fea(:,q+W), fea(:,q+W+1)] (128 f32 = 512 B), a
    single int16 gather row-index per (depth, pixel), and 4 folded corner
    weights (bilinear * valid * view_weight / C).
  - Device (per core, per depth-plane, per 2048-pixel chunk): one SWDGE
    dma_gather pulls 2048 x 512B corner-packs from HBM into SBUF in
    pixel-partitioned layout [128 pix, 16, 4*32]; DVE multiplies by the
    (resident, pre-transposed+replicated) ref features, tensor_reduce(X)
    sums over C, multiplies by corner weights, tensor_reduce(X) sums the
    4 corners -> [128 pix, 16] plane-chunk output.  No TensorE needed.
  - Host: un-permute, sum partials over the 4 views, divide by weight sum.

Self-contained: shapes hardcoded for the nn_DI_MVS problem instance.
"""
import numpy as np

B, V, C, H, W = 2, 5, 32, 128, 160
D = 48
HW = H * W
NCORES = 8
CHUNK = 2048             # pixels per dma_gather call
NCHUNKS = HW // CHUNK    # 10
PAD = W + 1              # index shift so clamped corner bases stay >= 0
NZ = HW + W + 1          # padded Z4 row count
ELEM = 4 * C             # 128 values per gathered row
DATA_F16 = True          # gather/ref data dtype (f16 halves HBM traffic)

_PROGRAM_CACHE = {}


# ----------------------------------------------------------------- host math
def _fold(proj):
    out = proj[0].copy()
    out[:3, :4] = (proj[1][:3, :3] @ proj[0][:3, :4]).astype(np.float32)
    return out


def _host_fields(features, proj_matrices, depth_values, view_weights):
    """Per (b,v) core: corner-base gather index + 4 folded corner weights."""
    ys, xs = np.meshgrid(np.arange(H, dtype=np.float32),
                         np.arange(W, dtype=np.float32), indexing='ij')
    grid = np.stack([xs.ravel(), ys.ravel(), np.ones(HW, dtype=np.float32)], 0)

    cores = []
    for b in range(B):
        ref_p = _fold(proj_matrices[b, 0])
        ref_p_inv = np.linalg.inv(ref_p.astype(np.float64)).astype(np.float32)
        for v in range(1, V):
            proj = (_fold(proj_matrices[b, v]).astype(np.float64)
                    @ ref_p_inv.astype(np.float64)).astype(np.float32)
            rot, trans = proj[:3, :3], proj[:3, 3]
            rot_xyz = rot.astype(np.float32) @ grid
            dep = depth_values[b].astype(np.float32)
            pxyz = (rot_xyz[:, None, :] * dep[None, :, None]
                    + trans[:, None, None]).astype(np.float32)
            px = (pxyz[0] / pxyz[2]).astype(np.float32)
            py = (pxyz[1] / pxyz[2]).astype(np.float32)
            x0 = np.floor(px)
            y0 = np.floor(py)
            wx = px - x0
            wy = py - y0
            vw = view_weights[b, v - 1].reshape(HW)

            # corner-base row index into the padded Z4 image
            x0c = np.clip(x0, -1, W - 1)
            y0c = np.clip(y0, -1, H - 1)
            idx = (y0c * W + x0c + PAD).astype(np.int32)     # (D, HW) in [0, NZ)

            wt4 = np.empty((4, D, HW), dtype=np.float32)
            corners = [(x0, y0, (1 - wx) * (1 - wy)),
                       (x0 + 1, y0, wx * (1 - wy)),
                       (x0, y0 + 1, (1 - wx) * wy),
                       (x0 + 1, y0 + 1, wx * wy)]
            for k, (xi, yi, wk) in enumerate(corners):
                valid = ((xi >= 0) & (xi <= W - 1) & (yi >= 0) & (yi <= H - 1))
                # a clamped base shifts which Z4 slot holds the corner's value;
                # those corners always have weight 0, so slot mismatch is fine.
                wt4[k] = (wk * valid).astype(np.float32) * vw[None, :] / np.float32(C)
            cores.append((b, v, idx, wt4))
    return cores


def _build_z4(src):
    """src: (C, HW) f32 -> padded corner-packed image (NZ, 4C) f32."""
    q = np.arange(NZ, dtype=np.int64) - PAD
    z4 = np.empty((NZ, 4, C), dtype=np.float32)
    for s, off in enumerate((0, 1, W, W + 1)):
        qi = np.clip(q + off, 0, HW - 1)
        z4[:, s, :] = src[:, qi].T
    return z4.reshape(NZ, 4 * C)


def _pack_core_inputs(features, cores):
    in_maps = []
    for (b, v, idx, wt4) in cores:
        src = features[b, v].reshape(C, HW).astype(np.float32)
        ref = features[b, 0].reshape(C, HW).astype(np.float32)
        dt = np.float16 if DATA_F16 else np.float32
        z4 = _build_z4(src).astype(dt)            # (NZ, 128)
        # ref transposed + duplicated x4 corners: (HW, 128)
        reft4 = np.tile(ref.T, (1, 4)).astype(dt)

        # idx tensor [D, NCHUNKS, 128, CHUNK//16] int16: wrapped in 16
        # partitions (j%16, j//16) and replicated to all 8 cores.
        blk = idx.reshape(D, NCHUNKS, CHUNK // 16, 16).astype(np.int16)
        wrap = blk.transpose(0, 1, 3, 2)          # (D, NCHUNKS, 16, 128)
        idx_t = np.tile(wrap, (1, 1, 8, 1))       # (D, NCHUNKS, 128, 128)

        # wts tensor [D, NCHUNKS, 128, 16*4]: [pixel%128, (i, corner)]
        wt_t = (wt4.transpose(1, 2, 0)            # (D, HW, 4)
                .reshape(D, NCHUNKS, 16, 128, 4)  # (d, ch, i, p, k)
                .transpose(0, 1, 3, 2, 4)         # (d, ch, p, i, k)
                .reshape(D, NCHUNKS, 128, 64)
                .astype(np.float16))

        in_maps.append({
            "z4": z4,
            "reft4": reft4,
            "idx": np.ascontiguousarray(idx_t),
            "wts": np.ascontiguousarray(wt_t),
        })
    return in_maps


# ------------------------------------------------------------- bass program
def _build_program():
    import concourse.bacc as bacc
    import concourse.tile as tile
    import concourse.mybir as mybir

    nc = bacc.Bacc("TRN2", target_bir_lowering=False, debug=False,
                   num_devices=NCORES, num_swdge_queues=4)
    f32 = mybir.dt.float32
    f16 = mybir.dt.float16 if DATA_F16 else mybir.dt.float32
    i16 = mybir.dt.int16

    z4_d = nc.dram_tensor("z4", [NZ, ELEM], f16, kind="ExternalInput")
    reft4_d = nc.dram_tensor("reft4", [HW, ELEM], f16, kind="ExternalInput")
    idx_d = nc.dram_tensor("idx", [D, NCHUNKS, 128, CHUNK // 16], i16,
                           kind="ExternalInput")
    wts_d = nc.dram_tensor("wts", [D, NCHUNKS, 128, 64], f16,
                           kind="ExternalInput")
    out_d = nc.dram_tensor("out", [D, NCHUNKS, 128, 16], f32,
                           kind="ExternalOutput")

    with tile.TileContext(nc) as tc:
        with (
            tc.tile_pool(name="big", bufs=1) as big,
            tc.tile_pool(name="gat", bufs=6) as gat,
            tc.tile_pool(name="idxp", bufs=8) as idxp,
            tc.tile_pool(name="wtp", bufs=8) as wtp,
            tc.tile_pool(name="crp", bufs=6) as crp,
            tc.tile_pool(name="outp", bufs=6) as outp,
        ):
            # resident ref: [128, NCHUNKS*16*128] with dst[p, (ch*16+i)*128+c]
            # = reft4[ch*2048 + i*128 + p, c]
            refsb = big.tile([128, HW // 128 * ELEM], f16)
            ref_src = reft4_d.ap().rearrange("(blk p) e -> p blk e", p=128)
            nc.sync.dma_start(
                refsb[:].rearrange("p (blk e) -> p blk e", e=ELEM), ref_src
            )

            z4_ap = z4_d.ap()
            gq = 0

            for d in range(D):
                for ch in range(NCHUNKS):
                    idxt = idxp.tile([128, CHUNK // 16], i16)
                    nc.sync.dma_start(idxt[:], idx_d.ap()[d, ch])
                    wtt = wtp.tile([128, 64], f16)
                    nc.sync.dma_start(wtt[:], wts_d.ap()[d, ch])

                    g = gat.tile([128, (CHUNK // 128) * ELEM], f16)
                    # SWDGE descriptor ring holds 1024 descs; split the
                    # 2048-row gather into two 1024-row calls.
                    half = CHUNK // 2
                    for h in range(2):
                        nc.gpsimd.dma_gather(
                            g[:, h * (half // 128) * ELEM:
                              (h + 1) * (half // 128) * ELEM]
                            .rearrange("p (i e) -> p i e", e=ELEM),
                            z4_ap,
                            idxt[:, h * (half // 16):(h + 1) * (half // 16)],
                            num_idxs=half,
                            num_idxs_reg=half,
                            elem_size=ELEM,
                            queue_num=gq % 4,
                        )
                        gq += 1
                    nc.vector.tensor_mul(
                        g[:], g[:],
                        refsb[:, ch * (CHUNK // 128) * ELEM:
                              (ch + 1) * (CHUNK // 128) * ELEM],
                    )
                    # reduce over C (innermost 32) with a 2x-mode f16 add
                    # tree instead of the 1x-mode tensor_reduce
                    ngrp = (CHUNK // 128) * 4          # 64 (i, corner) groups
                    src = g[:].rearrange("p (s c) -> p s c", c=C)
                    hw_ = C
                    while hw_ > 1:
                        h2 = hw_ // 2
                        dstt = crp.tile([128, ngrp * h2], f16, tag=f"tr{h2}")
                        dst = dstt[:].rearrange("p (s c) -> p s c", c=h2)
                        nc.vector.tensor_add(dst, src[:, :, 0:h2],
                                             src[:, :, h2:hw_])
                        src = dst
                        hw_ = h2
                    cr = crp.tile([128, 64], f16)
                    nc.vector.tensor_mul(
                        cr[:], src.rearrange("p s c -> p (s c)"), wtt[:])
                    # sum the 4 corners: two more f16 adds
                    c4 = cr[:].rearrange("p (i k) -> p i k", k=4)
                    c2t = crp.tile([128, 32], f16)
                    c2 = c2t[:].rearrange("p (i k) -> p i k", k=2)
                    nc.vector.tensor_add(c2, c4[:, :, 0:2], c4[:, :, 2:4])
                    outt = outp.tile([128, 16], f32)
                    nc.vector.tensor_add(
                        outt[:].rearrange("p (i k) -> p i k", k=1),
                        c2[:, :, 0:1], c2[:, :, 1:2])
                    nc.sync.dma_start(out_d.ap()[d, ch], outt[:])

    nc.compile()
    return nc


def _get_program():
    if "nc" not in _PROGRAM_CACHE:
        _PROGRAM_CACHE["nc"] = _build_program()
    return _PROGRAM_CACHE["nc"]


# -------------------------------------------------------------------- runner
def _run(inputs, trace=False):
    from concourse.bass_utils import run_bass_kernel_spmd

    features = np.asarray(inputs["features"], dtype=np.float32)
    proj_matrices = np.asarray(inputs["proj_matrices"], dtype=np.float32)
    depth_values = np.asarray(inputs["depth_values"], dtype=np.float32)
    view_weights = np.asarray(inputs["view_weights"], dtype=np.float32)

    cores = _host_fields(features, proj_matrices, depth_values, view_weights)
    in_maps = _pack_core_inputs(features, cores)
    nc = _get_program()

    res = run_bass_kernel_spmd(nc, in_maps, core_ids=list(range(NCORES)),
                               trace=trace)
    # out [D, NCHUNKS, 128, 16] -> [D, HW] with pixel = ch*2048 + i*128 + p
    partials = [
        res.results[i]["out"].transpose(0, 1, 3, 2).reshape(D, HW)
        for i in range(NCORES)
    ]

    out = np.empty((B, 1, D, H, W), dtype=np.float32)
    for b in range(B):
        vol = np.zeros((D, HW), dtype=np.float32)
        wsum = np.full((HW,), 1e-5, dtype=np.float32)
        for v in range(1, V):
            vol = vol + partials[b * 4 + (v - 1)]
            wsum = wsum + view_weights[b, v - 1].reshape(HW)
        out[b, 0] = (vol / wsum[None, :]).reshape(D, H, W)
    return out, res


def kernel(**inputs) -> np.ndarray:
    out, _ = _run(inputs, trace=False)
    return out

